# revision 19
# baseline (speedup 1.0000x reference)
"""Trainium2 Bass kernel for nn_AttentionWithContext (B=8, D=256, N=2048).

Data-parallel over batch: one batch element per NeuronCore (8 cores).

Math (per batch b, derived from the reference):
    h   = x[b].T @ W.T                       (N, D)
    sim = h @ h.T                            (N, N)   -- never materialized!
    C   = (m * sim) @ h ; only C @ a3 is ever used, so with w3 = h @ a3:
    s_jC[i] = sum_d h[i,:] . v[i,:],  v = (m .* sim) ... collapses to
    v[i,:] = sum_j m[i,j] * (w3[j] * h[j,:])          (mask-weighted matmul)
    s_j = h @ a2 + s_jC ; s_i = h @ a1
    scores = leaky_relu(s_i[:,None] + s_j[None,:], 0.2), masked by adj, softmax rows.

Key tricks:
  * v' = mT.T @ (w3 .* h)_hi/lo (exact bf16 splits; mask is exactly bf16) + rank-1 a2 rows
  * s_j row-dot fused via scalar_tensor_tensor accum_out
  * scores built on PE as a rank-5 matmul from exact bf16 splits of s_i/s_j
  * row max bound via masked log-sum-exp (matmul with exp(beta*s_j)): M_i >= true max
  * masking via (t2 - (M-SHIFT))*mask then exp(. - SHIFT): masked lanes -> exp(-SHIFT) == 0
"""
import numpy as np
import ml_dtypes
from contextlib import ExitStack

import concourse.bass as bass
import concourse.tile as tile
from concourse import bacc, mybir
from concourse.bass_utils import run_bass_kernel_spmd
from concourse.masks import make_identity

B, D, N = 8, 256, 2048
P = 128
NT = N // P   # 16
DB = D // P   # 2
NCORES = 8
SHIFT = 200.0
BETA = 0.2           # banded-lse exponent scale
NBANDS = 64          # number of lse bands
DELTA = 160.0        # band spacing; band k covers maskedmax in [ref_k-175, ref_k]
QTHR = float(np.exp(-35.0))   # discard bands with q below this (Ln LUT range)
BIGB = 1.0e5
QFLOOR = 1.0e-30     # keeps Ln input finite; discarded bands don't matter
DEBUG = bool(int(__import__("os").environ.get("K_DEBUG", "0")))

F32 = mybir.dt.float32
BF16 = mybir.dt.bfloat16
AF = mybir.ActivationFunctionType
OP = mybir.AluOpType


def _bcast_ap(ap: bass.AP, parts: int) -> bass.AP:
    """Partition-broadcast view of a 1-partition AP (stride-0 partition dim)."""
    return bass.AP(tensor=ap.tensor, offset=ap.offset, ap=[[0, parts]] + list(ap.ap[1:]))


def _emit(nc, tc, ctx, xb, wt, a1, a2h, a2l, a3, kline, kdrop, dcol, mT, msc, out):
    const = ctx.enter_context(tc.tile_pool(name="const", bufs=1))
    cols = ctx.enter_context(tc.tile_pool(name="cols", bufs=1))
    rows = ctx.enter_context(tc.tile_pool(name="rows", bufs=1))
    big = ctx.enter_context(tc.tile_pool(name="big", bufs=1))

    # ---- constants / small loads -------------------------------------------
    ident = const.tile([P, P], F32)
    make_identity(nc, ident[:])
    ones_k1 = const.tile([1, P], BF16)
    nc.vector.memset(ones_k1[:], 1.0)
    shiftneg = const.tile([P, 1], F32)
    nc.vector.memset(shiftneg[:], -SHIFT)

    a3_bc = const.tile([P, D], F32)
    nc.sync.dma_start(out=a3_bc[:], in_=_bcast_ap(a3.ap(), P))
    kline_bc = const.tile([P, NBANDS], F32)
    nc.sync.dma_start(out=kline_bc[:], in_=_bcast_ap(kline.ap(), P))
    kdrop_bc = const.tile([P, NBANDS], F32)
    nc.sync.dma_start(out=kdrop_bc[:], in_=_bcast_ap(kdrop.ap(), P))
    dcol_sb = const.tile([P, NT], F32)
    nc.sync.dma_start(out=dcol_sb[:], in_=dcol.ap())
    a1_sb = const.tile([P, DB, 1], F32)
    nc.sync.dma_start(out=a1_sb[:], in_=a1.ap().rearrange("(kb p) o -> p kb o", p=P))
    a2h_sb = const.tile([1, D], BF16)
    nc.sync.dma_start(out=a2h_sb[:], in_=a2h.ap())
    a2l_sb = const.tile([1, D], BF16)
    nc.sync.dma_start(out=a2l_sb[:], in_=a2l.ap())
    wt_sb = const.tile([P, DB, D], F32)
    nc.sync.dma_start(out=wt_sb[:], in_=wt.ap().rearrange("(kb p) d -> p kb d", p=P))

    # ---- big persistent SBUF tensors ---------------------------------------
    hT_sb = big.tile([P, DB, N], F32)       # hT[d, n] (2 MB)
    h_sb = big.tile([P, NT, D], F32)        # h[n, d]  (2 MB)
    hw_hi = big.tile([P, NT, D], BF16)      # (w3 .* h) hi split (1 MB)
    hw_lo = big.tile([P, NT, D], BF16)      # lo split (1 MB)

    # per-row-tile column vectors (each (128, NT) = node index t*128+p)
    si_col = cols.tile([P, NT], F32)
    sj_col = cols.tile([P, NT], F32)
    w3_col = cols.tile([P, NT], F32)
    preM_col = cols.tile([P, NT], F32)
    M_col = cols.tile([P, NT], F32)
    Mb_col = cols.tile([P, NT], F32)
    smax_bc = cols.tile([P, 1], F32)
    qfloor = cols.tile([P, 1], F32)
    nc.vector.memset(qfloor[:], QFLOOR)
    smax = cols.tile([1, 1], F32)

    # Row-layout operands for the rank-5 score matmul. All row-vector math is
    # done in (NT, P) = (16, 128) layout (cheap: 16 partitions, FD=128), then
    # DMA'd into the packed [5, N] operands row by row (DMA can address
    # arbitrary partition bases; compute engines cannot).
    lhsT_tile = rows.tile([5, N], BF16)     # rows: si_hi, si_lo, 1, 1, 1
    rhs_tile = rows.tile([5, N], BF16)      # rows: 1, 1, sj_hi, sj_mid, sj_lo
    ones16 = rows.tile([NT, P], BF16)
    nc.vector.memset(ones16[:], 1.0)
    sjT_sb = rows.tile([NT, P], F32)
    siT_sb = rows.tile([NT, P], F32)
    r1_16 = rows.tile([NT, P], F32)
    r2_16 = rows.tile([NT, P], F32)
    hi32_16 = rows.tile([NT, P], F32)
    sih_16 = rows.tile([NT, P], BF16)
    sil_16 = rows.tile([NT, P], BF16)
    sjh_16 = rows.tile([NT, P], BF16)
    sjm_16 = rows.tile([NT, P], BF16)
    sjl_16 = rows.tile([NT, P], BF16)

    def _fill_row(dst_tile, row, src16):
        """DMA a (NT, P) tile into one partition-row of a [5, N] operand."""
        for t in range(NT):
            nc.sync.dma_start(out=dst_tile[row:row + 1, t * P:(t + 1) * P],
                              in_=src16[t:t + 1, :])

    _fill_row(lhsT_tile, 2, ones16)
    _fill_row(lhsT_tile, 3, ones16)
    _fill_row(lhsT_tile, 4, ones16)
    _fill_row(rhs_tile, 0, ones16)
    _fill_row(rhs_tile, 1, ones16)

    with tc.tile_pool(name="mTp", bufs=1) as mTp, \
         tc.tile_pool(name="xp", bufs=1) as xp, \
         tc.tile_pool(name="scr", bufs=2) as scr, \
         tc.tile_pool(name="psA", bufs=2, space="PSUM") as psA, \
         tc.tile_pool(name="psS", bufs=2, space="PSUM") as psS, \
         tc.tile_pool(name="psR", bufs=1, space="PSUM") as psR:

        mT_sb = mTp.tile([P, NT, N], BF16)  # mT[j, i] by j-tile (8 MB)
        nc.sync.dma_start(out=mT_sb[:], in_=mT.ap().rearrange("(J p) i -> p J i", p=P))
        x_sb = xp.tile([P, DB, N], F32)
        nc.sync.dma_start(out=x_sb[:], in_=xb.ap().rearrange("(kb p) n -> p kb n", p=P))

        # ---- B: hT = W @ x[b]  (fp32) --------------------------------------
        for db in range(DB):
            for c in range(4):
                pt = psA.tile([P, 512], F32, tag="pb")
                for kb in range(DB):
                    nc.tensor.matmul(
                        pt[:],
                        lhsT=wt_sb[:, kb, db * P:(db + 1) * P],
                        rhs=x_sb[:, kb, c * 512:(c + 1) * 512],
                        start=(kb == 0), stop=(kb == DB - 1))
                nc.scalar.copy(out=hT_sb[:, db, c * 512:(c + 1) * 512], in_=pt[:])

        # ---- C: h (transpose), w3, hw splits -------------------------------
        for I in range(NT):
            ph = psS.tile([P, D], F32, tag="ps")
            for db in range(DB):
                nc.tensor.transpose(ph[:, db * P:(db + 1) * P],
                                    hT_sb[:, db, I * P:(I + 1) * P], ident[:])
            nc.scalar.copy(out=h_sb[:, I, :], in_=ph[:])
            s1 = scr.tile([P, D], F32, tag="scr")
            nc.vector.scalar_tensor_tensor(
                out=s1[:], in0=ph[:], scalar=0.0, in1=a3_bc[:],
                op0=OP.add, op1=OP.mult, accum_out=w3_col[:, I:I + 1])
            hwf = scr.tile([P, D], F32, tag="hwf")
            nc.vector.tensor_scalar_mul(hwf[:], h_sb[:, I, :], w3_col[:, I:I + 1])
            nc.vector.tensor_copy(out=hw_hi[:, I, :], in_=hwf[:])
            nc.vector.tensor_tensor(out=hw_lo[:, I, :], in0=hwf[:],
                                    in1=hw_hi[:, I, :], op=OP.subtract)

        # ---- D: s_i (column layout + transposed rows) ----------------------
        pq = psR.tile([P, NT], F32, tag="pq")
        for I in range(NT):
            for db in range(DB):
                nc.tensor.matmul(pq[:, I:I + 1], lhsT=hT_sb[:, db, I * P:(I + 1) * P],
                                 rhs=a1_sb[:, db, :], start=(db == 0), stop=(db == DB - 1))
        nc.vector.tensor_copy(out=si_col[:], in_=pq[:])
        pt1 = psR.tile([NT, P], F32, tag="pt2")
        nc.tensor.transpose(pt1[:], si_col[:], ident[:])
        nc.scalar.copy(out=siT_sb[:], in_=pt1[:])
        # si 2-way split -> lhsT_tile rows 0/1
        nc.vector.tensor_copy(out=sih_16[:], in_=siT_sb[:])
        nc.vector.tensor_copy(out=hi32_16[:], in_=sih_16[:])
        nc.vector.tensor_tensor(out=sil_16[:], in0=siT_sb[:],
                                in1=hi32_16[:], op=OP.subtract)
        _fill_row(lhsT_tile, 0, sih_16)
        _fill_row(lhsT_tile, 1, sil_16)

        # ---- E: v' = mT.T @ hw (+ rank-1 a2) ; s_j via fused row-dot -------
        for I in range(NT):
            pv = psS.tile([P, D], F32, tag="ps")
            for J in range(NT):
                lh = mT_sb[:, J, I * P:(I + 1) * P]
                nc.tensor.matmul(pv[:], lhsT=lh, rhs=hw_hi[:, J, :],
                                 start=(J == 0), stop=False)
                nc.tensor.matmul(pv[:], lhsT=lh, rhs=hw_lo[:, J, :],
                                 start=False, stop=False)
            nc.tensor.matmul(pv[:], lhsT=ones_k1[:], rhs=a2h_sb[:],
                             start=False, stop=False)
            nc.tensor.matmul(pv[:], lhsT=ones_k1[:], rhs=a2l_sb[:],
                             start=False, stop=True)
            s2 = scr.tile([P, D], F32, tag="scr")
            nc.vector.scalar_tensor_tensor(
                out=s2[:], in0=pv[:], scalar=0.0, in1=h_sb[:, I, :],
                op0=OP.add, op1=OP.mult, accum_out=sj_col[:, I:I + 1])

        # ---- F: s_j rows, smax, splits, X ----------------------------------
        pt2 = psR.tile([NT, P], F32, tag="pt2")
        nc.tensor.transpose(pt2[:], sj_col[:], ident[:])
        nc.scalar.copy(out=sjT_sb[:], in_=pt2[:])
        nc.gpsimd.tensor_reduce(out=smax[:], in_=sj_col[:],
                                axis=mybir.AxisListType.XYZWC, op=OP.max)
        smax_dram = nc.dram_tensor("smax_scratch", [1, 1], F32)
        nc.sync.dma_start(out=smax_dram.ap(), in_=smax[:])
        nc.sync.dma_start(out=smax_bc[:], in_=_bcast_ap(smax_dram.ap(), P))
        # 3-way split of s_j -> rhs_tile rows 2/3/4
        nc.vector.tensor_copy(out=sjh_16[:], in_=sjT_sb[:])
        nc.vector.tensor_copy(out=hi32_16[:], in_=sjh_16[:])
        nc.vector.tensor_tensor(out=r1_16[:], in0=sjT_sb[:], in1=hi32_16[:],
                                op=OP.subtract)
        nc.vector.tensor_copy(out=sjm_16[:], in_=r1_16[:])
        nc.vector.tensor_copy(out=hi32_16[:], in_=sjm_16[:])
        nc.vector.tensor_tensor(out=r2_16[:], in0=r1_16[:], in1=hi32_16[:],
                                op=OP.subtract)
        nc.vector.tensor_copy(out=sjl_16[:], in_=r2_16[:])
        _fill_row(rhs_tile, 2, sjh_16)
        _fill_row(rhs_tile, 3, sjm_16)
        _fill_row(rhs_tile, 4, sjl_16)

        # ---- G: banded masked log-sum-exp row-max bound --------------------
        # Band k reference ref_k = smax - k*DELTA. X[j, k] = exp(BETA *
        # min(s_j[j] - ref_k, 0)). q[i, k] = sum_j m[i,j] X[j, k] via PE.
        # est[i, k] = ln(q)/BETA - k*DELTA  (+ smax later); bands with
        # q < THR are discarded (Ln LUT is only accurate down to ~e^-40).
        # maskedmax_i <= smax + max_k est[i, k] <= maskedmax_i + ln(N)/BETA.
        if DEBUG:
            rmax_col = cols.tile([P, NT], F32)
            q_dbg = cols.tile([P, NT, NBANDS], F32)
        bsj_col = cols.tile([P, NT], F32)
        nc.vector.tensor_scalar(out=bsj_col[:], in0=sj_col[:], scalar1=smax_bc[:],
                                scalar2=BETA, op0=OP.subtract, op1=OP.mult)
        X_b = cols.tile([P, NT, NBANDS], BF16)
        yb = cols.tile([P, NBANDS], F32)
        for J in range(NT):
            nc.vector.tensor_scalar(out=yb[:], in0=kline_bc[:],
                                    scalar1=bsj_col[:, J:J + 1], scalar2=0.0,
                                    op0=OP.add, op1=OP.min)
            nc.scalar.activation(X_b[:, J, :], yb[:], AF.Exp, bias=0.0, scale=1.0)
        for I in range(NT):
            pqb = psS.tile([P, NBANDS], F32, tag="ps")
            for J in range(NT):
                nc.tensor.matmul(pqb[:, 0:NBANDS], lhsT=mT_sb[:, J, I * P:(I + 1) * P],
                                 rhs=X_b[:, J, :], start=(J == 0), stop=(J == NT - 1))
            # scores keep the diagonal (mT zeroes it): q += diag(adj)_i * X[i]
            qf_b = scr.tile([P, NBANDS], F32, tag="qf")
            nc.vector.scalar_tensor_tensor(
                out=qf_b[:], in0=X_b[:, I, :], scalar=dcol_sb[:, I:I + 1],
                in1=pqb[:, 0:NBANDS], op0=OP.mult, op1=OP.add)
            ind_b = scr.tile([P, NBANDS], F32, tag="scr")
            nc.vector.tensor_scalar(out=ind_b[:], in0=qf_b[:], scalar1=QTHR,
                                    scalar2=None, op0=OP.is_ge)
            lnq_b = scr.tile([P, NBANDS], F32, tag="hwf")
            nc.scalar.activation(lnq_b[:], qf_b[:], AF.Ln, bias=qfloor[:],
                                 scale=1.0)
            est_b = scr.tile([P, NBANDS], F32, tag="scr")
            nc.vector.scalar_tensor_tensor(
                out=est_b[:], in0=lnq_b[:], scalar=1.0 / BETA, in1=kdrop_bc[:],
                op0=OP.mult, op1=OP.add)
            estm_b = scr.tile([P, NBANDS], F32, tag="hwf")
            nc.vector.scalar_tensor_tensor(
                out=estm_b[:], in0=est_b[:], scalar=BIGB, in1=ind_b[:],
                op0=OP.add, op1=OP.mult)
            rmax = scr.tile([P, 1], F32, tag="rm")
            nc.vector.tensor_reduce(out=rmax[:], in_=estm_b[:],
                                    axis=mybir.AxisListType.X, op=OP.max)
            # preM = s_i + smax + (rowmax - BIGB)
            nc.vector.tensor_scalar(out=rmax[:], in0=rmax[:], scalar1=-BIGB,
                                    scalar2=smax_bc[:], op0=OP.add, op1=OP.add)
            if DEBUG:
                nc.vector.tensor_copy(out=rmax_col[:, I:I + 1], in_=rmax[:])
                nc.vector.tensor_copy(out=q_dbg[:, I, :], in_=qf_b[:])
            nc.vector.tensor_tensor(out=preM_col[:, I:I + 1], in0=rmax[:],
                                    in1=si_col[:, I:I + 1], op=OP.add)
        nc.scalar.activation(M_col[:], preM_col[:], AF.Prelu, bias=0.0, scale=1.0,
                             alpha=0.2)
        nc.vector.tensor_scalar_add(Mb_col[:], M_col[:], -SHIFT)

        if DEBUG:
            for name, t in [("d_sj", sj_col), ("d_si", si_col), ("d_w3", w3_col),
                            ("d_preM", preM_col), ("d_M", M_col)]:
                dt = nc.dram_tensor(name, [P, NT], F32, kind="ExternalOutput")
                nc.sync.dma_start(out=dt.ap(), in_=t[:])
            dsm = nc.dram_tensor("d_smax", [P, 1], F32, kind="ExternalOutput")
            nc.sync.dma_start(out=dsm.ap(), in_=smax_bc[:])
            drm = nc.dram_tensor("d_rmax", [P, NT], F32, kind="ExternalOutput")
            nc.sync.dma_start(out=drm.ap(), in_=rmax_col[:])
            dqb = nc.dram_tensor("d_qb", [P, NT, NBANDS], F32, kind="ExternalOutput")
            nc.sync.dma_start(out=dqb.ap(), in_=q_dbg[:])
            dxb = nc.dram_tensor("d_Xb", [P, NT, NBANDS], F32, kind="ExternalOutput")
            xbf = cols.tile([P, NT, NBANDS], F32)
            nc.vector.tensor_copy(out=xbf[:], in_=X_b[:])
            nc.sync.dma_start(out=dxb.ap(), in_=xbf[:])

    # ---- H: score phase (big pools freed; PSUM all ours) -------------------
    with tc.tile_pool(name="mscp", bufs=3) as mscp, \
         tc.tile_pool(name="work", bufs=2) as work, \
         tc.tile_pool(name="dens", bufs=4) as dens, \
         tc.tile_pool(name="psT", bufs=4, space="PSUM") as psT:
        for I in range(NT):
            msc_t = mscp.tile([P, N], BF16, tag="msc")
            nc.sync.dma_start(out=msc_t[:], in_=msc.ap()[I * P:(I + 1) * P, :])
            t2 = work.tile([P, N], F32, tag="t2")
            for half in range(2):
                pt = psT.tile([P, 1024], F32, tag="pt")
                for c in range(2):
                    nc.tensor.matmul(
                        pt[:, c * 512:(c + 1) * 512],
                        lhsT=lhsT_tile[:, I * P:(I + 1) * P],
                        rhs=rhs_tile[:, half * 1024 + c * 512: half * 1024 + (c + 1) * 512],
                        start=True, stop=True)
                nc.scalar.activation(t2[:, half * 1024:(half + 1) * 1024], pt[:],
                                     AF.Prelu, bias=0.0, scale=1.0, alpha=0.2)
            u2 = work.tile([P, N], F32, tag="u2")
            nc.vector.scalar_tensor_tensor(
                out=u2[:], in0=t2[:], scalar=Mb_col[:, I:I + 1], in1=msc_t[:],
                op0=OP.subtract, op1=OP.mult)
            e_t = work.tile([P, N], F32, tag="e")
            den = dens.tile([P, 1], F32, tag="den")
            nc.scalar.activation(e_t[:], u2[:], AF.Exp, bias=shiftneg[:], scale=1.0,
                                 accum_out=den[:])
            rcp = dens.tile([P, 1], F32, tag="rcp")
            nc.vector.reciprocal(out=rcp[:], in_=den[:])
            o_t = work.tile([P, N], F32, tag="o")
            nc.vector.tensor_scalar_mul(o_t[:], e_t[:], rcp[:])
            nc.sync.dma_start(out=out.ap()[I * P:(I + 1) * P, :], in_=o_t[:])


def _build():
    nc = bacc.Bacc("TRN2", target_bir_lowering=False, debug=False)
    xb = nc.dram_tensor("xb", [D, N], F32, kind="ExternalInput")
    wt = nc.dram_tensor("wt", [D, D], F32, kind="ExternalInput")
    a1 = nc.dram_tensor("a1", [D, 1], F32, kind="ExternalInput")
    a2h = nc.dram_tensor("a2h", [1, D], BF16, kind="ExternalInput")
    a2l = nc.dram_tensor("a2l", [1, D], BF16, kind="ExternalInput")
    a3 = nc.dram_tensor("a3", [1, D], F32, kind="ExternalInput")
    kline = nc.dram_tensor("kline", [1, NBANDS], F32, kind="ExternalInput")
    dcol = nc.dram_tensor("dcol", [P, NT], F32, kind="ExternalInput")
    kdrop = nc.dram_tensor("kdrop", [1, NBANDS], F32, kind="ExternalInput")
    mT = nc.dram_tensor("mT", [N, N], BF16, kind="ExternalInput")
    msc = nc.dram_tensor("msc", [N, N], BF16, kind="ExternalInput")
    out = nc.dram_tensor("out", [N, N], F32, kind="ExternalOutput")
    with tile.TileContext(nc) as tc, ExitStack() as ctx:
        _emit(nc, tc, ctx, xb, wt, a1, a2h, a2l, a3, kline, kdrop, dcol, mT, msc, out)
    nc.compile()
    return nc


_NC_CACHE = None


def _get_nc():
    global _NC_CACHE
    if _NC_CACHE is None:
        _NC_CACHE = _build()
    return _NC_CACHE


def make_in_maps(x, adj, W, a):
    """Host-side prep: shard over batch, build masks/splits (all numpy)."""
    x = np.asarray(x, dtype=np.float32)
    adj = np.asarray(adj)
    W = np.asarray(W, dtype=np.float32)
    a = np.asarray(a, dtype=np.float32)

    wt = np.ascontiguousarray(W.T)
    a1 = np.ascontiguousarray(a[:D].reshape(D, 1))
    a2 = a[D:2 * D].reshape(1, D)
    a2h = a2.astype(ml_dtypes.bfloat16)
    a2l = (a2 - a2h.astype(np.float32)).astype(ml_dtypes.bfloat16)
    a3 = np.ascontiguousarray(a[2 * D:].reshape(1, D))

    kline_np = (BETA * DELTA * np.arange(NBANDS, dtype=np.float32)).reshape(1, NBANDS)
    kdrop_np = (-DELTA * np.arange(NBANDS, dtype=np.float32)).reshape(1, NBANDS)
    dcol_np = np.ascontiguousarray(
        (np.diagonal(adj) != 0).astype(np.float32).reshape(NT, P).T)

    adj_nz = (adj != 0)
    msc = adj_nz.astype(ml_dtypes.bfloat16)
    mTm = adj_nz.T.copy()
    np.fill_diagonal(mTm, False)
    mT = mTm.astype(ml_dtypes.bfloat16)

    in_maps = []
    for b in range(NCORES):
        in_maps.append({
            "xb": np.ascontiguousarray(x[b]),
            "wt": wt, "a1": a1, "a2h": a2h, "a2l": a2l, "a3": a3,
            "kline": kline_np, "kdrop": kdrop_np, "dcol": dcol_np,
            "mT": mT, "msc": msc,
        })
    return in_maps


def kernel(x, adj, W, a, _trace=False, _trace_kwargs=None):
    nc = _get_nc()
    in_maps = make_in_maps(x, adj, W, a)
    kw = {}
    if _trace:
        kw["trace"] = True
        if _trace_kwargs:
            kw.update(_trace_kwargs)
    res = run_bass_kernel_spmd(nc, in_maps, core_ids=list(range(NCORES)), **kw)
    outp = np.stack([res.results[b]["out"] for b in range(NCORES)], axis=0)
    if _trace:
        return outp, res
    return outp
